# revision 19
# baseline (speedup 1.0000x reference)
"""Trainium2 Bass kernel for a dense transformer block (B=4, N=1024, D=1024,
H=16, Dh=64, MLP 4x), distributed over 8 NeuronCores with ZERO collectives.

Sharding: core c handles batch b = c//2, sequence half = c%2 (512 query
rows).  K/V are computed for the batch's full 1024-token sequence on both
cores of a pair.  The sequence is rotated per-core so the core's own 512
rows are always rows 0..511 of its input — attention is permutation-
invariant over keys, so all 8 cores run one identical SPMD program.

v2 changes vs v1 (451us baseline):
- Weights are cast to bf16 ON THE HOST and DMA'd as bf16: all device-side
  weight casts (165us gpsimd + 49us DVE busy) and their pipeline stalls
  are gone, and weight DMA bytes are halved.
- K is stored PACKED ([even-head dh | odd-head dh] on the partition axis,
  no zero padding): score matmuls use K=64 contractions at tile rows 0/64.
- Softmax exp runs as [128,1024] activations over score pairs written to
  two adjacent PSUM banks (one ACT instruction per 2 head-parities per
  key block): ~2x fewer ACT instructions on the attention critical path.
- Softmax denominators: DVE reciprocal + bf16 ones-row matmul broadcast.
- Matmul streams are kept dense so the PE p-state ramps to 2.4GHz
  (measured: 216ns steady-state per 512-wide bf16 matmul vs 454ns in v1).
"""

import numpy as np

import bass_rust
import concourse.bass as bass
import concourse.mybir as mybir
import concourse.tile as tile
from concourse.masks import make_identity

F32 = mybir.dt.float32
BF16 = mybir.dt.bfloat16
AF = mybir.ActivationFunctionType
ALU = mybir.AluOpType

P = 128
D = 1024
S = 1024          # full sequence (per batch)
SO = 512          # own rows per core
H = 16
DH = 64
F = 4096
EPS = 1e-5
N_CORES = 8

ND = D // P       # 8   d tiles
NS = S // P       # 8   full-seq tiles
NSO = SO // P     # 4   own-seq tiles
NF = F // P       # 32  ff tiles
VC = 66           # VN free cols: 64 dh + ones + pad


# --------------------------------------------------------------------------
# Workaround: this compiler build supports only ONE semaphore wait per
# instruction.  Move excess waits onto fresh NOPs inserted just before the
# offending instruction on the same engine.
# --------------------------------------------------------------------------
_counter = [0]


def _split_multiwaits(nc):
    nsplit = 0
    for fn in nc.m.functions:
        for blk in fn.blocks:
            il = list(blk.instructions)
            out = []
            changed = False
            for inst in il:
                si = inst.sync_info
                if si is not None and len(si.on_wait) > 1:
                    waits = list(si.on_wait)
                    for w in waits[:-1]:
                        _counter[0] += 1
                        nop = mybir.InstNoOp(
                            name=f"I-waitsplit-{_counter[0]}", ins=[], outs=[]
                        )
                        nop.engine = inst.engine
                        nop.sync_info = bass_rust.SyncInfo(on_wait=[w], on_update=[])
                        out.append(nop)
                        nc.register_instruction(nop, overwrite=True)
                    inst.sync_info = bass_rust.SyncInfo(
                        on_wait=[waits[-1]], on_update=list(si.on_update)
                    )
                    changed = True
                    nsplit += 1
                out.append(inst)
            if changed:
                blk.instructions = out
    return nsplit


def _vec_tile(nc, pool, ext, n, eng=None):
    """Load a host-pre-arranged [128, n] dram tensor into SBUF."""
    t = pool.tile([P, n], F32, name=ext.name + "_sb")
    (eng or nc.sync).dma_start(out=t[:], in_=ext[:])
    return t


def _bcast_tile(nc, pool, ext, n, eng=None):
    """Load a host-pre-broadcast [128, n] dram tensor into SBUF."""
    t = pool.tile([P, n], F32, name=ext.name + "_bc")
    (eng or nc.sync).dma_start(out=t[:], in_=ext[:])
    return t


def build():
    nc = bass.Bass(name="tfblock")

    # Constant vectors arrive pre-arranged from the host: [128, n] "tile
    # column" layouts and pre-broadcast [128, D] bias rows.  The on-device
    # rearranging DMAs ((o p)->p o gathers, stride-0 broadcasts) take
    # 7-14us just to ISSUE and clogged the sync queue.
    x_ext = nc.declare_dram_parameter("x", [S, D], F32, isOutput=False)
    ln1_w = nc.declare_dram_parameter("ln1_w", [P, ND], F32, isOutput=False)
    ln1_b = nc.declare_dram_parameter("ln1_b", [P, ND], F32, isOutput=False)
    Wq_e = nc.declare_dram_parameter("Wq", [D, D], BF16, isOutput=False)
    bq_e = nc.declare_dram_parameter("bq", [P, ND], F32, isOutput=False)
    Wk_e = nc.declare_dram_parameter("Wk", [D, D], BF16, isOutput=False)
    bk_e = nc.declare_dram_parameter("bk", [P, ND], F32, isOutput=False)
    Wv_e = nc.declare_dram_parameter("Wv", [D, D], BF16, isOutput=False)
    bv_e = nc.declare_dram_parameter("bv", [P, D], F32, isOutput=False)
    Wo_e = nc.declare_dram_parameter("Wo", [D, D], BF16, isOutput=False)
    bo_e = nc.declare_dram_parameter("bo", [P, D], F32, isOutput=False)
    ln2_w = nc.declare_dram_parameter("ln2_w", [P, ND], F32, isOutput=False)
    ln2_b = nc.declare_dram_parameter("ln2_b", [P, ND], F32, isOutput=False)
    Wfc_e = nc.declare_dram_parameter("Wfc", [D, F], BF16, isOutput=False)
    bfc_e = nc.declare_dram_parameter("bfc", [P, NF], F32, isOutput=False)
    Wp_e = nc.declare_dram_parameter("Wproj", [F, D], BF16, isOutput=False)
    bp_e = nc.declare_dram_parameter("bproj", [P, D], F32, isOutput=False)
    out_ext = nc.declare_dram_parameter("out", [SO, D], F32, isOutput=True)

    def ln_tile(lnp, src_ap, hn_out, eps_t, tag):
        """LayerNorm stats on DVE + apply on ACT: hn_out = (src-mu)*rstd."""
        stats = lnp.tile([P, 2, 6], F32, tag=tag + "_st")
        for g in range(2):
            nc.vector.bn_stats(out=stats[:, g, :], in_=src_ap[:, g * 512 : (g + 1) * 512])
        mv = lnp.tile([P, 2], F32, tag=tag + "_mv")
        nc.vector.bn_aggr(out=mv[:], in_=stats[:])
        lnv = lnp.tile([P, 1], F32, tag=tag + "_sd")
        nc.scalar.activation(out=lnv[:], in_=mv[:, 1:2], func=AF.Ln, bias=eps_t[:])
        rstd = lnp.tile([P, 1], F32, tag=tag + "_rs")
        nc.scalar.activation(out=rstd[:], in_=lnv[:], func=AF.Exp, scale=-0.5)
        nb = lnp.tile([P, 1], F32, tag=tag + "_nb")
        nc.vector.tensor_scalar(nb[:], mv[:, 0:1], rstd[:], -1.0, ALU.mult, ALU.mult)
        nc.scalar.activation(
            out=hn_out, in_=src_ap, func=AF.Identity, bias=nb[:], scale=rstd[:]
        )

    with tile.TileContext(nc) as tc:
        from contextlib import ExitStack

        with ExitStack() as top:
            consts = top.enter_context(tc.tile_pool(name="consts", bufs=1))
            persist = top.enter_context(tc.tile_pool(name="persist", bufs=1))

            eps_t = consts.tile([P, 1], F32, name="eps")
            nc.vector.memset(eps_t[:], EPS)
            # bf16 ones-row for the denominator broadcast matmul
            e0 = consts.tile([P, P], BF16, name="e0")
            nc.vector.memset(e0[:], 0.0)
            nc.vector.memset(e0[0:1, :], 1.0)
            ident = consts.tile([P, P], BF16, name="ident")
            make_identity(nc, ident[:])
            # denominator reciprocals: row 0 live, rows 1.. stay zero
            rec = consts.tile([P, 2, SO], BF16, name="rec")
            nc.gpsimd.memset(rec[:], 0.0)

            # x1N survives into the MLP phases; everything attention-scoped
            # lives in `alife` (closed right after the Wo residual).
            x1N = persist.tile([P, NSO, D], F32, name="x1N")
            h2T = persist.tile([P, ND, SO], BF16, name="h2T")

            alife_cm = tc.tile_pool(name="alife", bufs=1)
            alife = alife_cm.__enter__()
            xN_own = alife.tile([P, NSO, D], F32, name="xN_own")
            for st in range(NSO):
                nc.sync.dma_start(
                    out=xN_own[:, st, :], in_=x_ext[st * P : (st + 1) * P, :]
                )
            ln1w_t = _vec_tile(nc, consts, ln1_w, ND)
            ln1b_t = _vec_tile(nc, consts, ln1_b, ND)
            QT = alife.tile([P, ND, SO], BF16, name="QT")
            KT = alife.tile([P, ND, S], BF16, name="KT")
            VN = alife.tile([P, NS, H, VC], BF16, name="VN")
            OT = alife.tile([P, ND, SO], BF16, name="OT")
            nc.vector.memset(VN[:, :, :, DH : DH + 1], 1.0)
            nc.vector.memset(VN[:, :, :, DH + 1 :], 0.0)

            # hT and Wv survive into the attention phase: V heads 8..15
            # are produced inside the attention loop (PE slack under the
            # ACT-bound softmax), first needed by head-pair j=4.
            phAB_cm = tc.tile_pool(name="phAB", bufs=1)
            phAB = phAB_cm.__enter__()
            hT_own = phAB.tile([P, ND, SO], BF16, name="hT_own")
            hT_oth = phAB.tile([P, ND, SO], BF16, name="hT_oth")
            Wv_sb = phAB.tile([P, ND, D], BF16, name="Wv_sb")

            def v_group(st, oh, ps_ap):
                hTx = hT_own if st < NSO else hT_oth
                st4 = st % NSO
                for kt in range(ND):
                    nc.tensor.matmul(
                        ps_ap,
                        hTx[:, kt, st4 * P : (st4 + 1) * P],
                        Wv_sb[:, kt, oh * SO : (oh + 1) * SO],
                        start=(kt == 0),
                        stop=(kt == ND - 1),
                    )
                nc.vector.tensor_tensor(
                    VN[:, st, oh * 8 : (oh + 1) * 8, 0:DH],
                    ps_ap.rearrange("p (h e) -> p h e", h=8),
                    _bv[0][:, oh * SO : (oh + 1) * SO].rearrange(
                        "p (h e) -> p h e", h=8
                    ),
                    ALU.add,
                )

            _bv = [None]

            # ------------------------- phase A: LN1, h^T, Q/K/V
            with ExitStack() as phA:
                xothp = phA.enter_context(tc.tile_pool(name="xoth", bufs=1))
                x_oth = xothp.tile([P, NSO, D], F32, name="x_oth")
                for st in range(NSO):
                    nc.sync.dma_start(
                        out=x_oth[:, st, :],
                        in_=x_ext[(NSO + st) * P : (NSO + st + 1) * P, :],
                    )

                lnpool = phA.enter_context(tc.tile_pool(name="ln1", bufs=2))
                hnp = phA.enter_context(tc.tile_pool(name="hn1", bufs=2))
                psT = phA.enter_context(tc.tile_pool(name="psT", bufs=4, space="PSUM"))
                psQ = phA.enter_context(tc.tile_pool(name="psQ", bufs=3, space="PSUM"))

                wqp = phA.enter_context(tc.tile_pool(name="wqkv", bufs=1))
                Wq_sb = wqp.tile([P, ND, D], BF16, name="Wq_sb")
                Wk_sb = wqp.tile([P, ND, D], BF16, name="Wk_sb")
                for w_ext, w_sb in ((Wq_e, Wq_sb), (Wk_e, Wk_sb), (Wv_e, Wv_sb)):
                    for kt in range(ND):
                        eng = nc.gpsimd if kt % 2 == 0 else nc.sync
                        eng.dma_start(
                            out=w_sb[:, kt, :], in_=w_ext[kt * P : (kt + 1) * P, :]
                        )

                tcb = [0]

                def trans_copyback(dst_ap, src_ap, w_ap, b_ap):
                    # alternate DVE / ACT for psum->sbuf LN-scale copybacks
                    # (gpsimd has no PSUM access)
                    if tcb[0] % 2 == 0:
                        nc.vector.tensor_scalar(
                            dst_ap, src_ap, w_ap, b_ap, ALU.mult, ALU.add
                        )
                    else:
                        nc.scalar.activation(
                            out=dst_ap, in_=src_ap, func=AF.Identity,
                            bias=b_ap, scale=w_ap,
                        )
                    tcb[0] += 1

                def ln1_for(st):
                    src = xN_own[:, st, :] if st < NSO else x_oth[:, st - NSO, :]
                    hn = hnp.tile([P, D], BF16, tag="hn")
                    ln_tile(lnpool, src, hn[:], eps_t, "l1")
                    hTx = hT_own if st < NSO else hT_oth
                    st4 = st % NSO
                    for dt in range(ND):
                        pst = psT.tile([P, P], BF16, tag="ps_t")
                        nc.tensor.transpose(
                            pst[:], hn[:, dt * P : (dt + 1) * P], ident[:]
                        )
                        trans_copyback(
                            hTx[:, dt, st4 * P : (st4 + 1) * P],
                            pst[:],
                            ln1w_t[:, dt : dt + 1],
                            ln1b_t[:, dt : dt + 1],
                        )

                for st in range(NSO):
                    ln1_for(st)

                # Q for own rows (hT_own complete; Wq arrives ~12us in)
                bq_t = _vec_tile(nc, consts, bq_e, ND)
                bk_t = _vec_tile(nc, consts, bk_e, ND)
                bv_bc = _bcast_tile(nc, consts, bv_e, D)
                _bv[0] = bv_bc
                for ot in range(ND):
                    ps = psQ.tile([P, SO], F32, tag="ps_a")
                    for kt in range(ND):
                        nc.tensor.matmul(
                            ps[:],
                            Wq_sb[:, kt, ot * P : (ot + 1) * P],
                            hT_own[:, kt, :],
                            start=(kt == 0),
                            stop=(kt == ND - 1),
                        )
                    nc.vector.tensor_scalar(
                        QT[:, ot, :], ps[:], bq_t[:, ot : ot + 1], None, ALU.add
                    )

                for st in range(NSO, NS):
                    ln1_for(st)

                # K for the full sequence; psum block ot covers heads
                # 2ot (partitions 0:64) and 2ot+1 (64:128) == packed KT rows
                for ot in range(ND):
                    for sh in range(2):
                        hTx = hT_own if sh == 0 else hT_oth
                        ps = psQ.tile([P, SO], F32, tag="ps_a")
                        for kt in range(ND):
                            nc.tensor.matmul(
                                ps[:],
                                Wk_sb[:, kt, ot * P : (ot + 1) * P],
                                hTx[:, kt, :],
                                start=(kt == 0),
                                stop=(kt == ND - 1),
                            )
                        if (ot + sh) % 2 == 0:
                            nc.vector.tensor_scalar(
                                KT[:, ot, sh * SO : (sh + 1) * SO],
                                ps[:],
                                bk_t[:, ot : ot + 1],
                                None,
                                ALU.add,
                            )
                        else:
                            nc.scalar.activation(
                                out=KT[:, ot, sh * SO : (sh + 1) * SO],
                                in_=ps[:],
                                func=AF.Identity,
                                bias=bk_t[:, ot : ot + 1],
                            )

                # V natural ([seq, d]) with bias, +ones column for
                # denominators.  oh=0 (heads 0..7) is produced here; oh=1
                # (heads 8..15, first needed by j=4) is interleaved into the
                # attention loop where the PE otherwise idles waiting on exp.
                for st in range(NS):
                    for oh in range(2):
                        ps = psQ.tile([P, SO], F32, tag="ps_a")
                        v_group(st, oh, ps[:])

            # ------------------------- phase B: attention
            with ExitStack() as phB:
                wop = phB.enter_context(tc.tile_pool(name="wo", bufs=1))
                Wo_sb = wop.tile([P, ND, D], BF16, name="Wo_sb")
                for kt in range(ND):
                    nc.gpsimd.dma_start(
                        out=Wo_sb[:, kt, :], in_=Wo_e[kt * P : (kt + 1) * P, :]
                    )
                bo_bc = _bcast_tile(nc, wop, bo_e, D, eng=nc.gpsimd)

                # pre-bias the residual with bo (x + bo), in place, on gpsimd
                for st in range(NSO):
                    nc.gpsimd.tensor_tensor(
                        xN_own[:, st, :], xN_own[:, st, :], bo_bc[:], ALU.add
                    )

                phB1 = phB.enter_context(ExitStack())
                attn = phB1.enter_context(tc.tile_pool(name="attn", bufs=3))
                ps_s = phB1.enter_context(
                    tc.tile_pool(name="ps_s", bufs=2, space="PSUM")
                )
                ps_o = phB1.enter_context(
                    tc.tile_pool(name="ps_o", bufs=4, space="PSUM")
                )

                def normalize_recip(po_a, po_b):
                    # batch both parities into one multi-pass DVE reciprocal
                    # (cost is free-size driven, so partitions are free);
                    # partition bases must be 0/64-aligned.
                    den2 = attn.tile([P, SO], F32, tag="den2")
                    nc.vector.tensor_copy(out=den2[0:1, :], in_=po_a[DH : DH + 1, :])
                    nc.vector.tensor_copy(
                        out=den2[DH : DH + 1, :], in_=po_b[DH : DH + 1, :]
                    )
                    r2 = attn.tile([P, SO], F32, tag="r2")
                    nc.vector.reciprocal(out=r2[:], in_=den2[:])
                    nc.vector.tensor_copy(out=rec[0:1, 0, :], in_=r2[0:1, :])
                    nc.vector.tensor_copy(out=rec[0:1, 1, :], in_=r2[DH : DH + 1, :])

                def normalize_half(j, po, par, psb):
                    nc.tensor.matmul(
                        psb[:, par, :], e0[:], rec[:, par, :], start=True, stop=True
                    )
                    bcast = attn.tile([DH, SO], F32, tag="bcast")
                    nc.vector.tensor_copy(out=bcast[:], in_=psb[0:DH, par, :])
                    nc.vector.tensor_tensor(
                        OT[par * DH : (par + 1) * DH, j, :],
                        po[0:DH, :],
                        bcast[:],
                        ALU.mult,
                    )

                def attn_j(j, pending):
                    # software-pipelined: scores(kb) + exp(kb) issue BEFORE
                    # AV(kb-1), so the PE reaches scores(kb+1) without
                    # waiting on exp(kb) and the ACT exp stream never
                    # starves (exp is the attention-phase floor).
                    po_a = ps_o.tile([P, SO], F32, tag="po")
                    po_b = ps_o.tile([P, SO], F32, tag="po")
                    pas = {}
                    for kb in range(NS):
                        pss = ps_s.tile([P, 2, SO], F32, tag="ps_s")
                        nc.tensor.matmul(
                            pss[:, 0, :],
                            KT[0:DH, j, kb * P : (kb + 1) * P],
                            QT[0:DH, j, :],
                            start=True,
                            stop=True,
                        )
                        nc.tensor.matmul(
                            pss[:, 1, :],
                            KT[DH:P, j, kb * P : (kb + 1) * P],
                            QT[DH:P, j, :],
                            start=True,
                            stop=True,
                        )
                        pa = attn.tile([P, 2, SO], BF16, tag="probs")
                        nc.scalar.activation(
                            out=pa[:].rearrange("p a b -> p (a b)"),
                            in_=pss[:].rearrange("p a b -> p (a b)"),
                            func=AF.Exp,
                            scale=0.125,
                        )
                        pas[kb] = pa

                        def av_pair(k):
                            nc.tensor.matmul(
                                po_a[0:VC, :],
                                VN[:, k, 2 * j, :],
                                pas[k][:, 0, :],
                                start=(k == 0),
                                stop=(k == NS - 1),
                            )
                            nc.tensor.matmul(
                                po_b[0:VC, :],
                                VN[:, k, 2 * j + 1, :],
                                pas[k][:, 1, :],
                                start=(k == 0),
                                stop=(k == NS - 1),
                            )

                        if kb > 0:
                            av_pair(kb - 1)
                        if kb == 1 and pending is not None:
                            normalize_recip(pending[1], pending[2])
                        if kb == 5 and pending is not None:
                            # normalize j-1 in the shadow of j's score
                            # stream; the recip chain (started at kb1) is
                            # ready by then.
                            psb = ps_s.tile([P, 2, SO], F32, tag="ps_s")
                            normalize_half(pending[0], pending[1], 0, psb)
                            normalize_half(pending[0], pending[2], 1, psb)
                    av_pair(NS - 1)
                    return (j, po_a, po_b)

                pending = None
                for j in range(H // 2):
                    pending = attn_j(j, pending)
                normalize_recip(pending[1], pending[2])
                psb = ps_s.tile([P, 2, SO], F32, tag="ps_s")
                normalize_half(pending[0], pending[1], 0, psb)
                normalize_half(pending[0], pending[2], 1, psb)
                phB1.close()

                # Wo projection, NATURAL output, fused residual:
                # x1[q, d] = (x + bo)[q, d] + sum_j OT[:,j,q].T @ Wo[j, d]
                # Wo projection with LN2 + h^T interleaved per query block:
                # LN2(qb) runs on DVE/ACT while Wo(qb+1) occupies the PE.
                psD = phB.enter_context(tc.tile_pool(name="psD", bufs=3, space="PSUM"))
                psT2 = phB.enter_context(tc.tile_pool(name="psT2", bufs=4, space="PSUM"))
                hnp2 = phB.enter_context(tc.tile_pool(name="hn2", bufs=2))
                lnp2 = phB.enter_context(tc.tile_pool(name="ln2p", bufs=2))
                ln2w_t = _vec_tile(nc, consts, ln2_w, ND)
                ln2b_t = _vec_tile(nc, consts, ln2_b, ND)
                tcb2 = [0]

                def ln2_trans(qb, hn):
                    # PE transposes + copybacks, emitted one qb late so the
                    # in-order PE queue never blocks on the LN2 chain
                    for dt in range(ND):
                        pst = psT2.tile([P, P], BF16, tag="ps_t2")
                        nc.tensor.transpose(
                            pst[:], hn[:, dt * P : (dt + 1) * P], ident[:]
                        )
                        if tcb2[0] % 2 == 0:
                            nc.vector.tensor_scalar(
                                h2T[:, dt, qb * P : (qb + 1) * P],
                                pst[:],
                                ln2w_t[:, dt : dt + 1],
                                ln2b_t[:, dt : dt + 1],
                                ALU.mult,
                                ALU.add,
                            )
                        else:
                            nc.scalar.activation(
                                out=h2T[:, dt, qb * P : (qb + 1) * P],
                                in_=pst[:],
                                func=AF.Identity,
                                bias=ln2b_t[:, dt : dt + 1],
                                scale=ln2w_t[:, dt : dt + 1],
                            )
                        tcb2[0] += 1

                hn_prev = None
                for qb in range(NSO):
                    for dh in range(2):
                        ps = psD.tile([P, SO], F32, tag="ps_d")
                        for kt in range(ND):
                            nc.tensor.matmul(
                                ps[:],
                                OT[:, kt, qb * P : (qb + 1) * P],
                                Wo_sb[:, kt, dh * SO : (dh + 1) * SO],
                                start=(kt == 0),
                                stop=(kt == ND - 1),
                            )
                        nc.vector.tensor_tensor(
                            x1N[:, qb, dh * SO : (dh + 1) * SO],
                            xN_own[:, qb, dh * SO : (dh + 1) * SO],
                            ps[:],
                            ALU.add,
                        )
                    hn = hnp2.tile([P, D], BF16, tag="hn2")
                    ln_tile(lnp2, x1N[:, qb, :], hn[:], eps_t, "l2")
                    if hn_prev is not None:
                        ln2_trans(qb - 1, hn_prev)
                    hn_prev = hn
                ln2_trans(NSO - 1, hn_prev)

            phAB_cm.__exit__(None, None, None)
            alife_cm.__exit__(None, None, None)

            # ------------------------- phase C/D/E: LN2, MLP fc, MLP proj
            with ExitStack() as phF:
                h2p = phF.enter_context(tc.tile_pool(name="h2p", bufs=1))
                wpp = phF.enter_context(tc.tile_pool(name="wpp", bufs=1))
                gtp = phF.enter_context(tc.tile_pool(name="gtp", bufs=1))
                psF = phF.enter_context(tc.tile_pool(name="psF", bufs=3, space="PSUM"))
                opool = phF.enter_context(tc.tile_pool(name="opool", bufs=3))

                bfc_t = _vec_tile(nc, consts, bfc_e, NF)
                bp_bc = _bcast_tile(nc, h2p, bp_e, D, eng=nc.gpsimd)
                # Wproj streams during LN2+fc on the gpsimd queue
                Wp_sb = wpp.tile([P, NF, D], BF16, name="Wp_sb")
                for fq in range(8):
                    nc.gpsimd.dma_start(
                        out=Wp_sb[:, fq * 4 : (fq + 1) * 4, :],
                        in_=Wp_e[fq * SO : (fq + 1) * SO, :].rearrange(
                            "(ft p) d -> p ft d", p=P
                        ),
                    )

                GT = gtp.tile([P, NF, SO], BF16, name="GT")

                # fc: stream Wfc in 4 quarters (ring bufs=2), G^T = gelu(.)
                wfc_dma = [nc.sync, nc.scalar]
                with tc.tile_pool(name="wfcq", bufs=2) as wfcq:
                    for q in range(4):
                        wfc = wfcq.tile([P, ND, 1024], BF16, tag="wfc")
                        for kt in range(ND):
                            wfc_dma[kt % 2].dma_start(
                                out=wfc[:, kt, :],
                                in_=Wfc_e[
                                    kt * P : (kt + 1) * P,
                                    q * 1024 : (q + 1) * 1024,
                                ],
                            )
                        for fl in range(8):
                            ft = q * 8 + fl
                            ps = psF.tile([P, SO], F32, tag="ps_f")
                            for kt in range(ND):
                                nc.tensor.matmul(
                                    ps[:],
                                    wfc[:, kt, fl * P : (fl + 1) * P],
                                    h2T[:, kt, :],
                                    start=(kt == 0),
                                    stop=(kt == ND - 1),
                                )
                            nc.scalar.activation(
                                out=GT[:, ft, :],
                                in_=ps[:],
                                func=AF.Gelu,
                                bias=bfc_t[:, ft : ft + 1],
                            )
                        if q == 0:
                            # pre-bias the residual with bproj, on gpsimd
                            for st in range(NSO):
                                nc.gpsimd.tensor_tensor(
                                    x1N[:, st, :], x1N[:, st, :], bp_bc[:], ALU.add
                                )

                # proj, NATURAL output, fused residual:
                # out[s, d] = (x1 + bproj)[s, d] + sum_ft GT[:,ft,s].T @ Wp[ft, d]
                for dh in range(2):
                    for qb in range(NSO):
                        ps = psF.tile([P, SO], F32, tag="ps_f")
                        for ft in range(NF):
                            nc.tensor.matmul(
                                ps[:],
                                GT[:, ft, qb * P : (qb + 1) * P],
                                Wp_sb[:, ft, dh * SO : (dh + 1) * SO],
                                start=(ft == 0),
                                stop=(ft == NF - 1),
                            )
                        of = opool.tile([P, SO], F32, tag="of")
                        nc.vector.tensor_tensor(
                            of[:],
                            x1N[:, qb, dh * SO : (dh + 1) * SO],
                            ps[:],
                            ALU.add,
                        )
                        nc.sync.dma_start(
                            out=out_ext[qb * P : (qb + 1) * P, dh * SO : (dh + 1) * SO],
                            in_=of[:],
                        )

    _split_multiwaits(nc)
    return nc


_NC_CACHE = None


def _get_nc():
    global _NC_CACHE
    if _NC_CACHE is None:
        _NC_CACHE = build()
    return _NC_CACHE


def make_in_maps(inputs):
    """Shard FULL inputs into per-core input maps (own rows rotated first).

    Weight matrices are rounded to bf16 on the host — the device consumes
    them directly with no cast instructions."""
    import ml_dtypes

    x = np.asarray(inputs["x"], dtype=np.float32)

    def cols(name):
        v = np.asarray(inputs[name], dtype=np.float32)
        return np.ascontiguousarray(v.reshape(-1, P).T)

    def brow(name):
        v = np.asarray(inputs[name], dtype=np.float32)
        return np.ascontiguousarray(np.broadcast_to(v, (P, v.shape[0])))

    bf16_names = ["Wq", "Wk", "Wv", "Wo", "Wfc", "Wproj"]
    shared = {n: cols(n) for n in
              ["ln1_w", "ln1_b", "bq", "bk", "ln2_w", "ln2_b", "bfc"]}
    shared.update({n: brow(n) for n in ["bv", "bo", "bproj"]})
    shared.update(
        {n: np.ascontiguousarray(
            np.asarray(inputs[n], dtype=np.float32).astype(ml_dtypes.bfloat16))
         for n in bf16_names}
    )
    in_maps = []
    for c in range(N_CORES):
        b, half = c // 2, c % 2
        xb = x[b]
        x_core = np.concatenate(
            [xb[half * SO : (half + 1) * SO], xb[(1 - half) * SO : (2 - half) * SO]],
            axis=0,
        )
        m = {"x": np.ascontiguousarray(x_core)}
        m.update(shared)
        in_maps.append(m)
    return in_maps


def kernel(**inputs) -> np.ndarray:
    from concourse.bass_utils import run_bass_kernel_spmd

    nc = _get_nc()
    in_maps = make_in_maps(inputs)
    res = run_bass_kernel_spmd(nc, in_maps, list(range(N_CORES)))
    B = 4
    out = np.empty((B, S, D), dtype=np.float32)
    for c in range(N_CORES):
        b, half = c // 2, c % 2
        out[b, half * SO : (half + 1) * SO] = res.results[c]["out"]
    return out


# revision 20
# speedup vs baseline: 1.0179x; 1.0179x over previous
"""Trainium2 Bass kernel for a dense transformer block (B=4, N=1024, D=1024,
H=16, Dh=64, MLP 4x), distributed over 8 NeuronCores with ZERO collectives.

Sharding: core c handles batch b = c//2, sequence half = c%2 (512 query
rows).  K/V are computed for the batch's full 1024-token sequence on both
cores of a pair.  The sequence is rotated per-core so the core's own 512
rows are always rows 0..511 of its input — attention is permutation-
invariant over keys, so all 8 cores run one identical SPMD program.

v2 changes vs v1 (451us baseline):
- Weights are cast to bf16 ON THE HOST and DMA'd as bf16: all device-side
  weight casts (165us gpsimd + 49us DVE busy) and their pipeline stalls
  are gone, and weight DMA bytes are halved.
- K is stored PACKED ([even-head dh | odd-head dh] on the partition axis,
  no zero padding): score matmuls use K=64 contractions at tile rows 0/64.
- Softmax exp runs as [128,1024] activations over score pairs written to
  two adjacent PSUM banks (one ACT instruction per 2 head-parities per
  key block): ~2x fewer ACT instructions on the attention critical path.
- Softmax denominators: DVE reciprocal + bf16 ones-row matmul broadcast.
- Matmul streams are kept dense so the PE p-state ramps to 2.4GHz
  (measured: 216ns steady-state per 512-wide bf16 matmul vs 454ns in v1).
"""

import numpy as np

import bass_rust
import concourse.bass as bass
import concourse.mybir as mybir
import concourse.tile as tile
from concourse.masks import make_identity

F32 = mybir.dt.float32
BF16 = mybir.dt.bfloat16
AF = mybir.ActivationFunctionType
ALU = mybir.AluOpType

P = 128
D = 1024
S = 1024          # full sequence (per batch)
SO = 512          # own rows per core
H = 16
DH = 64
F = 4096
EPS = 1e-5
N_CORES = 8

ND = D // P       # 8   d tiles
NS = S // P       # 8   full-seq tiles
NSO = SO // P     # 4   own-seq tiles
NF = F // P       # 32  ff tiles
VC = 66           # VN free cols: 64 dh + ones + pad


# --------------------------------------------------------------------------
# Workaround: this compiler build supports only ONE semaphore wait per
# instruction.  Move excess waits onto fresh NOPs inserted just before the
# offending instruction on the same engine.
# --------------------------------------------------------------------------
_counter = [0]


def _split_multiwaits(nc):
    nsplit = 0
    for fn in nc.m.functions:
        for blk in fn.blocks:
            il = list(blk.instructions)
            out = []
            changed = False
            for inst in il:
                si = inst.sync_info
                if si is not None and len(si.on_wait) > 1:
                    waits = list(si.on_wait)
                    for w in waits[:-1]:
                        _counter[0] += 1
                        nop = mybir.InstNoOp(
                            name=f"I-waitsplit-{_counter[0]}", ins=[], outs=[]
                        )
                        nop.engine = inst.engine
                        nop.sync_info = bass_rust.SyncInfo(on_wait=[w], on_update=[])
                        out.append(nop)
                        nc.register_instruction(nop, overwrite=True)
                    inst.sync_info = bass_rust.SyncInfo(
                        on_wait=[waits[-1]], on_update=list(si.on_update)
                    )
                    changed = True
                    nsplit += 1
                out.append(inst)
            if changed:
                blk.instructions = out
    return nsplit


def _vec_tile(nc, pool, ext, n, eng=None):
    """Load a host-pre-arranged [128, n] dram tensor into SBUF."""
    t = pool.tile([P, n], F32, name=ext.name + "_sb")
    (eng or nc.sync).dma_start(out=t[:], in_=ext[:])
    return t


def _bcast_tile(nc, pool, ext, n, eng=None):
    """Load a host-pre-broadcast [128, n] dram tensor into SBUF."""
    t = pool.tile([P, n], F32, name=ext.name + "_bc")
    (eng or nc.sync).dma_start(out=t[:], in_=ext[:])
    return t


def build():
    nc = bass.Bass(name="tfblock")

    # Constant vectors arrive pre-arranged from the host: [128, n] "tile
    # column" layouts and pre-broadcast [128, D] bias rows.  The on-device
    # rearranging DMAs ((o p)->p o gathers, stride-0 broadcasts) take
    # 7-14us just to ISSUE and clogged the sync queue.
    x_ext = nc.declare_dram_parameter("x", [S, D], F32, isOutput=False)
    ln1_w = nc.declare_dram_parameter("ln1_w", [P, ND], F32, isOutput=False)
    ln1_b = nc.declare_dram_parameter("ln1_b", [P, ND], F32, isOutput=False)
    Wq_e = nc.declare_dram_parameter("Wq", [D, D], BF16, isOutput=False)
    bq_e = nc.declare_dram_parameter("bq", [P, ND], F32, isOutput=False)
    Wk_e = nc.declare_dram_parameter("Wk", [D, D], BF16, isOutput=False)
    bk_e = nc.declare_dram_parameter("bk", [P, ND], F32, isOutput=False)
    Wv_e = nc.declare_dram_parameter("Wv", [D, D], BF16, isOutput=False)
    bv_e = nc.declare_dram_parameter("bv", [P, D], F32, isOutput=False)
    Wo_e = nc.declare_dram_parameter("Wo", [D, D], BF16, isOutput=False)
    bo_e = nc.declare_dram_parameter("bo", [P, D], F32, isOutput=False)
    ln2_w = nc.declare_dram_parameter("ln2_w", [P, ND], F32, isOutput=False)
    ln2_b = nc.declare_dram_parameter("ln2_b", [P, ND], F32, isOutput=False)
    Wfc_e = nc.declare_dram_parameter("Wfc", [D, F], BF16, isOutput=False)
    bfc_e = nc.declare_dram_parameter("bfc", [P, NF], F32, isOutput=False)
    Wp_e = nc.declare_dram_parameter("Wproj", [F, D], BF16, isOutput=False)
    bp_e = nc.declare_dram_parameter("bproj", [P, D], F32, isOutput=False)
    out_ext = nc.declare_dram_parameter("out", [SO, D], F32, isOutput=True)

    def ln_tile(lnp, src_ap, hn_out, eps_t, tag):
        """LayerNorm stats on DVE + apply on ACT: hn_out = (src-mu)*rstd."""
        stats = lnp.tile([P, 2, 6], F32, tag=tag + "_st")
        for g in range(2):
            nc.vector.bn_stats(out=stats[:, g, :], in_=src_ap[:, g * 512 : (g + 1) * 512])
        mv = lnp.tile([P, 2], F32, tag=tag + "_mv")
        nc.vector.bn_aggr(out=mv[:], in_=stats[:])
        lnv = lnp.tile([P, 1], F32, tag=tag + "_sd")
        nc.scalar.activation(out=lnv[:], in_=mv[:, 1:2], func=AF.Ln, bias=eps_t[:])
        rstd = lnp.tile([P, 1], F32, tag=tag + "_rs")
        nc.scalar.activation(out=rstd[:], in_=lnv[:], func=AF.Exp, scale=-0.5)
        nb = lnp.tile([P, 1], F32, tag=tag + "_nb")
        nc.vector.tensor_scalar(nb[:], mv[:, 0:1], rstd[:], -1.0, ALU.mult, ALU.mult)
        nc.scalar.activation(
            out=hn_out, in_=src_ap, func=AF.Identity, bias=nb[:], scale=rstd[:]
        )

    with tile.TileContext(nc) as tc:
        from contextlib import ExitStack

        with ExitStack() as top:
            consts = top.enter_context(tc.tile_pool(name="consts", bufs=1))
            persist = top.enter_context(tc.tile_pool(name="persist", bufs=1))

            eps_t = consts.tile([P, 1], F32, name="eps")
            nc.vector.memset(eps_t[:], EPS)
            # bf16 ones-row for the denominator broadcast matmul
            e0 = consts.tile([P, P], BF16, name="e0")
            nc.vector.memset(e0[:], 0.0)
            nc.vector.memset(e0[0:1, :], 1.0)
            ident = consts.tile([P, P], BF16, name="ident")
            make_identity(nc, ident[:])
            # denominator reciprocals: row 0 live, rows 1.. stay zero
            rec = consts.tile([P, 2, SO], BF16, name="rec")
            nc.gpsimd.memset(rec[:], 0.0)

            # x1N survives into the MLP phases; everything attention-scoped
            # lives in `alife` (closed right after the Wo residual).
            x1N = persist.tile([P, NSO, D], F32, name="x1N")
            h2T = persist.tile([P, ND, SO], BF16, name="h2T")
            wfc0 = persist.tile([P, ND, 1024], BF16, name="wfc0")

            alife_cm = tc.tile_pool(name="alife", bufs=1)
            alife = alife_cm.__enter__()
            xN_own = alife.tile([P, NSO, D], F32, name="xN_own")
            for st in range(NSO):
                nc.sync.dma_start(
                    out=xN_own[:, st, :], in_=x_ext[st * P : (st + 1) * P, :]
                )
            ln1w_t = _vec_tile(nc, consts, ln1_w, ND)
            ln1b_t = _vec_tile(nc, consts, ln1_b, ND)
            QT = alife.tile([P, ND, SO], BF16, name="QT")
            KT = alife.tile([P, ND, S], BF16, name="KT")
            VN = alife.tile([P, NS, H, VC], BF16, name="VN")
            OT = alife.tile([P, ND, SO], BF16, name="OT")
            nc.vector.memset(VN[:, :, :, DH : DH + 1], 1.0)
            nc.vector.memset(VN[:, :, :, DH + 1 :], 0.0)

            # hT and Wv survive into the attention phase: V heads 8..15
            # are produced inside the attention loop (PE slack under the
            # ACT-bound softmax), first needed by head-pair j=4.
            phAB_cm = tc.tile_pool(name="phAB", bufs=1)
            phAB = phAB_cm.__enter__()
            hT_own = phAB.tile([P, ND, SO], BF16, name="hT_own")
            hT_oth = phAB.tile([P, ND, SO], BF16, name="hT_oth")
            Wv_sb = phAB.tile([P, ND, D], BF16, name="Wv_sb")

            def v_group(st, oh, ps_ap):
                hTx = hT_own if st < NSO else hT_oth
                st4 = st % NSO
                for kt in range(ND):
                    nc.tensor.matmul(
                        ps_ap,
                        hTx[:, kt, st4 * P : (st4 + 1) * P],
                        Wv_sb[:, kt, oh * SO : (oh + 1) * SO],
                        start=(kt == 0),
                        stop=(kt == ND - 1),
                    )
                nc.vector.tensor_tensor(
                    VN[:, st, oh * 8 : (oh + 1) * 8, 0:DH],
                    ps_ap.rearrange("p (h e) -> p h e", h=8),
                    _bv[0][:, oh * SO : (oh + 1) * SO].rearrange(
                        "p (h e) -> p h e", h=8
                    ),
                    ALU.add,
                )

            _bv = [None]

            # ------------------------- phase A: LN1, h^T, Q/K/V
            with ExitStack() as phA:
                xothp = phA.enter_context(tc.tile_pool(name="xoth", bufs=1))
                x_oth = xothp.tile([P, NSO, D], F32, name="x_oth")
                for st in range(NSO):
                    nc.sync.dma_start(
                        out=x_oth[:, st, :],
                        in_=x_ext[(NSO + st) * P : (NSO + st + 1) * P, :],
                    )

                lnpool = phA.enter_context(tc.tile_pool(name="ln1", bufs=2))
                hnp = phA.enter_context(tc.tile_pool(name="hn1", bufs=2))
                psT = phA.enter_context(tc.tile_pool(name="psT", bufs=4, space="PSUM"))
                psQ = phA.enter_context(tc.tile_pool(name="psQ", bufs=3, space="PSUM"))

                wqp = phA.enter_context(tc.tile_pool(name="wqkv", bufs=1))
                Wq_sb = wqp.tile([P, ND, D], BF16, name="Wq_sb")
                Wk_sb = wqp.tile([P, ND, D], BF16, name="Wk_sb")
                for w_ext, w_sb in ((Wq_e, Wq_sb), (Wk_e, Wk_sb), (Wv_e, Wv_sb)):
                    for kt in range(ND):
                        eng = nc.gpsimd if kt % 2 == 0 else nc.sync
                        eng.dma_start(
                            out=w_sb[:, kt, :], in_=w_ext[kt * P : (kt + 1) * P, :]
                        )

                tcb = [0]

                def trans_copyback(dst_ap, src_ap, w_ap, b_ap):
                    # alternate DVE / ACT for psum->sbuf LN-scale copybacks
                    # (gpsimd has no PSUM access)
                    if tcb[0] % 2 == 0:
                        nc.vector.tensor_scalar(
                            dst_ap, src_ap, w_ap, b_ap, ALU.mult, ALU.add
                        )
                    else:
                        nc.scalar.activation(
                            out=dst_ap, in_=src_ap, func=AF.Identity,
                            bias=b_ap, scale=w_ap,
                        )
                    tcb[0] += 1

                def ln1_for(st):
                    src = xN_own[:, st, :] if st < NSO else x_oth[:, st - NSO, :]
                    hn = hnp.tile([P, D], BF16, tag="hn")
                    ln_tile(lnpool, src, hn[:], eps_t, "l1")
                    hTx = hT_own if st < NSO else hT_oth
                    st4 = st % NSO
                    for dt in range(ND):
                        pst = psT.tile([P, P], BF16, tag="ps_t")
                        nc.tensor.transpose(
                            pst[:], hn[:, dt * P : (dt + 1) * P], ident[:]
                        )
                        trans_copyback(
                            hTx[:, dt, st4 * P : (st4 + 1) * P],
                            pst[:],
                            ln1w_t[:, dt : dt + 1],
                            ln1b_t[:, dt : dt + 1],
                        )

                for st in range(NSO):
                    ln1_for(st)

                # Q for own rows (hT_own complete; Wq arrives ~12us in)
                bq_t = _vec_tile(nc, consts, bq_e, ND)
                bk_t = _vec_tile(nc, consts, bk_e, ND)
                bv_bc = _bcast_tile(nc, consts, bv_e, D)
                _bv[0] = bv_bc
                for ot in range(ND):
                    ps = psQ.tile([P, SO], F32, tag="ps_a")
                    for kt in range(ND):
                        nc.tensor.matmul(
                            ps[:],
                            Wq_sb[:, kt, ot * P : (ot + 1) * P],
                            hT_own[:, kt, :],
                            start=(kt == 0),
                            stop=(kt == ND - 1),
                        )
                    nc.vector.tensor_scalar(
                        QT[:, ot, :], ps[:], bq_t[:, ot : ot + 1], None, ALU.add
                    )

                for st in range(NSO, NS):
                    ln1_for(st)

                # K for the full sequence; psum block ot covers heads
                # 2ot (partitions 0:64) and 2ot+1 (64:128) == packed KT rows
                for ot in range(ND):
                    for sh in range(2):
                        hTx = hT_own if sh == 0 else hT_oth
                        ps = psQ.tile([P, SO], F32, tag="ps_a")
                        for kt in range(ND):
                            nc.tensor.matmul(
                                ps[:],
                                Wk_sb[:, kt, ot * P : (ot + 1) * P],
                                hTx[:, kt, :],
                                start=(kt == 0),
                                stop=(kt == ND - 1),
                            )
                        if (ot + sh) % 2 == 0:
                            nc.vector.tensor_scalar(
                                KT[:, ot, sh * SO : (sh + 1) * SO],
                                ps[:],
                                bk_t[:, ot : ot + 1],
                                None,
                                ALU.add,
                            )
                        else:
                            nc.scalar.activation(
                                out=KT[:, ot, sh * SO : (sh + 1) * SO],
                                in_=ps[:],
                                func=AF.Identity,
                                bias=bk_t[:, ot : ot + 1],
                            )

                # V natural ([seq, d]) with bias, +ones column for
                # denominators.  oh=0 (heads 0..7) is produced here; oh=1
                # (heads 8..15, first needed by j=4) is interleaved into the
                # attention loop where the PE otherwise idles waiting on exp.
                for st in range(NS):
                    for oh in range(2):
                        ps = psQ.tile([P, SO], F32, tag="ps_a")
                        v_group(st, oh, ps[:])

            # ------------------------- phase B: attention
            with ExitStack() as phB:
                # prefetch Wfc quarter 0 during attention (sync queue idle);
                # its buffer is top-level so the DMA never waits pool closes
                for kt in range(ND):
                    nc.sync.dma_start(
                        out=wfc0[:, kt, :],
                        in_=Wfc_e[kt * P : (kt + 1) * P, 0:1024],
                    )
                wop = phB.enter_context(tc.tile_pool(name="wo", bufs=1))
                Wo_sb = wop.tile([P, ND, D], BF16, name="Wo_sb")
                for kt in range(ND):
                    nc.gpsimd.dma_start(
                        out=Wo_sb[:, kt, :], in_=Wo_e[kt * P : (kt + 1) * P, :]
                    )
                bo_bc = _bcast_tile(nc, wop, bo_e, D, eng=nc.gpsimd)

                # pre-bias the residual with bo (x + bo), in place, on gpsimd
                for st in range(NSO):
                    nc.gpsimd.tensor_tensor(
                        xN_own[:, st, :], xN_own[:, st, :], bo_bc[:], ALU.add
                    )

                phB1 = phB.enter_context(ExitStack())
                attn = phB1.enter_context(tc.tile_pool(name="attn", bufs=3))
                ps_s = phB1.enter_context(
                    tc.tile_pool(name="ps_s", bufs=2, space="PSUM")
                )
                ps_o = phB1.enter_context(
                    tc.tile_pool(name="ps_o", bufs=4, space="PSUM")
                )

                def normalize_recip(po_a, po_b):
                    # batch both parities into one multi-pass DVE reciprocal
                    # (cost is free-size driven, so partitions are free);
                    # partition bases must be 0/64-aligned.
                    den2 = attn.tile([P, SO], F32, tag="den2")
                    nc.vector.tensor_copy(out=den2[0:1, :], in_=po_a[DH : DH + 1, :])
                    nc.vector.tensor_copy(
                        out=den2[DH : DH + 1, :], in_=po_b[DH : DH + 1, :]
                    )
                    r2 = attn.tile([P, SO], F32, tag="r2")
                    nc.vector.reciprocal(out=r2[:], in_=den2[:])
                    nc.vector.tensor_copy(out=rec[0:1, 0, :], in_=r2[0:1, :])
                    nc.vector.tensor_copy(out=rec[0:1, 1, :], in_=r2[DH : DH + 1, :])

                def normalize_half(j, po, par, psb):
                    nc.tensor.matmul(
                        psb[:, par, :], e0[:], rec[:, par, :], start=True, stop=True
                    )
                    bcast = attn.tile([DH, SO], F32, tag="bcast")
                    nc.vector.tensor_copy(out=bcast[:], in_=psb[0:DH, par, :])
                    nc.vector.tensor_tensor(
                        OT[par * DH : (par + 1) * DH, j, :],
                        po[0:DH, :],
                        bcast[:],
                        ALU.mult,
                    )

                def attn_j(j, pending):
                    # software-pipelined: scores(kb) + exp(kb) issue BEFORE
                    # AV(kb-1), so the PE reaches scores(kb+1) without
                    # waiting on exp(kb) and the ACT exp stream never
                    # starves (exp is the attention-phase floor).
                    po_a = ps_o.tile([P, SO], F32, tag="po")
                    po_b = ps_o.tile([P, SO], F32, tag="po")
                    pas = {}
                    for kb in range(NS):
                        pss = ps_s.tile([P, 2, SO], F32, tag="ps_s")
                        nc.tensor.matmul(
                            pss[:, 0, :],
                            KT[0:DH, j, kb * P : (kb + 1) * P],
                            QT[0:DH, j, :],
                            start=True,
                            stop=True,
                        )
                        nc.tensor.matmul(
                            pss[:, 1, :],
                            KT[DH:P, j, kb * P : (kb + 1) * P],
                            QT[DH:P, j, :],
                            start=True,
                            stop=True,
                        )
                        pa = attn.tile([P, 2, SO], BF16, tag="probs")
                        nc.scalar.activation(
                            out=pa[:].rearrange("p a b -> p (a b)"),
                            in_=pss[:].rearrange("p a b -> p (a b)"),
                            func=AF.Exp,
                            scale=0.125,
                        )
                        pas[kb] = pa

                        def av_pair(k):
                            nc.tensor.matmul(
                                po_a[0:VC, :],
                                VN[:, k, 2 * j, :],
                                pas[k][:, 0, :],
                                start=(k == 0),
                                stop=(k == NS - 1),
                            )
                            nc.tensor.matmul(
                                po_b[0:VC, :],
                                VN[:, k, 2 * j + 1, :],
                                pas[k][:, 1, :],
                                start=(k == 0),
                                stop=(k == NS - 1),
                            )

                        if kb > 0:
                            av_pair(kb - 1)
                        if kb == 1 and pending is not None:
                            normalize_recip(pending[1], pending[2])
                        if kb == 5 and pending is not None:
                            # normalize j-1 in the shadow of j's score
                            # stream; the recip chain (started at kb1) is
                            # ready by then.
                            psb = ps_s.tile([P, 2, SO], F32, tag="ps_s")
                            normalize_half(pending[0], pending[1], 0, psb)
                            normalize_half(pending[0], pending[2], 1, psb)
                    av_pair(NS - 1)
                    return (j, po_a, po_b)

                pending = None
                for j in range(H // 2):
                    pending = attn_j(j, pending)
                normalize_recip(pending[1], pending[2])
                psb = ps_s.tile([P, 2, SO], F32, tag="ps_s")
                normalize_half(pending[0], pending[1], 0, psb)
                normalize_half(pending[0], pending[2], 1, psb)
                phB1.close()

                # Wo projection, NATURAL output, fused residual:
                # x1[q, d] = (x + bo)[q, d] + sum_j OT[:,j,q].T @ Wo[j, d]
                # Wo projection with LN2 + h^T interleaved per query block:
                # LN2(qb) runs on DVE/ACT while Wo(qb+1) occupies the PE.
                psD = phB.enter_context(tc.tile_pool(name="psD", bufs=3, space="PSUM"))
                psT2 = phB.enter_context(tc.tile_pool(name="psT2", bufs=4, space="PSUM"))
                hnp2 = phB.enter_context(tc.tile_pool(name="hn2", bufs=2))
                lnp2 = phB.enter_context(tc.tile_pool(name="ln2p", bufs=2))
                ln2w_t = _vec_tile(nc, consts, ln2_w, ND)
                ln2b_t = _vec_tile(nc, consts, ln2_b, ND)
                tcb2 = [0]

                def ln2_trans(qb, hn):
                    # PE transposes + copybacks, emitted one qb late so the
                    # in-order PE queue never blocks on the LN2 chain
                    for dt in range(ND):
                        pst = psT2.tile([P, P], BF16, tag="ps_t2")
                        nc.tensor.transpose(
                            pst[:], hn[:, dt * P : (dt + 1) * P], ident[:]
                        )
                        if tcb2[0] % 2 == 0:
                            nc.vector.tensor_scalar(
                                h2T[:, dt, qb * P : (qb + 1) * P],
                                pst[:],
                                ln2w_t[:, dt : dt + 1],
                                ln2b_t[:, dt : dt + 1],
                                ALU.mult,
                                ALU.add,
                            )
                        else:
                            nc.scalar.activation(
                                out=h2T[:, dt, qb * P : (qb + 1) * P],
                                in_=pst[:],
                                func=AF.Identity,
                                bias=ln2b_t[:, dt : dt + 1],
                                scale=ln2w_t[:, dt : dt + 1],
                            )
                        tcb2[0] += 1

                hn_prev = None
                for qb in range(NSO):
                    for dh in range(2):
                        ps = psD.tile([P, SO], F32, tag="ps_d")
                        for kt in range(ND):
                            nc.tensor.matmul(
                                ps[:],
                                OT[:, kt, qb * P : (qb + 1) * P],
                                Wo_sb[:, kt, dh * SO : (dh + 1) * SO],
                                start=(kt == 0),
                                stop=(kt == ND - 1),
                            )
                        nc.vector.tensor_tensor(
                            x1N[:, qb, dh * SO : (dh + 1) * SO],
                            xN_own[:, qb, dh * SO : (dh + 1) * SO],
                            ps[:],
                            ALU.add,
                        )
                    hn = hnp2.tile([P, D], BF16, tag="hn2")
                    ln_tile(lnp2, x1N[:, qb, :], hn[:], eps_t, "l2")
                    if hn_prev is not None:
                        ln2_trans(qb - 1, hn_prev)
                    hn_prev = hn
                ln2_trans(NSO - 1, hn_prev)

            phAB_cm.__exit__(None, None, None)
            alife_cm.__exit__(None, None, None)

            # ------------------------- phase C/D/E: LN2, MLP fc, MLP proj
            with ExitStack() as phF:
                h2p = phF.enter_context(tc.tile_pool(name="h2p", bufs=1))
                wpp = phF.enter_context(tc.tile_pool(name="wpp", bufs=1))
                gtp = phF.enter_context(tc.tile_pool(name="gtp", bufs=1))
                psF = phF.enter_context(tc.tile_pool(name="psF", bufs=3, space="PSUM"))
                opool = phF.enter_context(tc.tile_pool(name="opool", bufs=3))

                bfc_t = _vec_tile(nc, consts, bfc_e, NF)
                bp_bc = _bcast_tile(nc, h2p, bp_e, D, eng=nc.gpsimd)
                # Wproj streams during LN2+fc on the gpsimd queue
                Wp_sb = wpp.tile([P, NF, D], BF16, name="Wp_sb")
                for fq in range(8):
                    nc.gpsimd.dma_start(
                        out=Wp_sb[:, fq * 4 : (fq + 1) * 4, :],
                        in_=Wp_e[fq * SO : (fq + 1) * SO, :].rearrange(
                            "(ft p) d -> p ft d", p=P
                        ),
                    )

                GT = gtp.tile([P, NF, SO], BF16, name="GT")

                # fc: stream Wfc in 4 quarters (ring bufs=2), G^T = gelu(.)
                wfc_dma = [nc.sync, nc.scalar]
                with tc.tile_pool(name="wfcq", bufs=2) as wfcq:
                    for q in range(4):
                        if q == 0:
                            wfc = wfc0
                        else:
                            wfc = wfcq.tile([P, ND, 1024], BF16, tag="wfc")
                            for kt in range(ND):
                                wfc_dma[kt % 2].dma_start(
                                    out=wfc[:, kt, :],
                                    in_=Wfc_e[
                                        kt * P : (kt + 1) * P,
                                        q * 1024 : (q + 1) * 1024,
                                    ],
                                )
                        for fl in range(8):
                            ft = q * 8 + fl
                            ps = psF.tile([P, SO], F32, tag="ps_f")
                            for kt in range(ND):
                                nc.tensor.matmul(
                                    ps[:],
                                    wfc[:, kt, fl * P : (fl + 1) * P],
                                    h2T[:, kt, :],
                                    start=(kt == 0),
                                    stop=(kt == ND - 1),
                                )
                            nc.scalar.activation(
                                out=GT[:, ft, :],
                                in_=ps[:],
                                func=AF.Gelu,
                                bias=bfc_t[:, ft : ft + 1],
                            )
                        if q == 0:
                            # pre-bias the residual with bproj, on gpsimd
                            for st in range(NSO):
                                nc.gpsimd.tensor_tensor(
                                    x1N[:, st, :], x1N[:, st, :], bp_bc[:], ALU.add
                                )

                # proj, NATURAL output, fused residual:
                # out[s, d] = (x1 + bproj)[s, d] + sum_ft GT[:,ft,s].T @ Wp[ft, d]
                for dh in range(2):
                    for qb in range(NSO):
                        ps = psF.tile([P, SO], F32, tag="ps_f")
                        for ft in range(NF):
                            nc.tensor.matmul(
                                ps[:],
                                GT[:, ft, qb * P : (qb + 1) * P],
                                Wp_sb[:, ft, dh * SO : (dh + 1) * SO],
                                start=(ft == 0),
                                stop=(ft == NF - 1),
                            )
                        of = opool.tile([P, SO], F32, tag="of")
                        nc.vector.tensor_tensor(
                            of[:],
                            x1N[:, qb, dh * SO : (dh + 1) * SO],
                            ps[:],
                            ALU.add,
                        )
                        nc.sync.dma_start(
                            out=out_ext[qb * P : (qb + 1) * P, dh * SO : (dh + 1) * SO],
                            in_=of[:],
                        )

    _split_multiwaits(nc)
    return nc


_NC_CACHE = None


def _get_nc():
    global _NC_CACHE
    if _NC_CACHE is None:
        _NC_CACHE = build()
    return _NC_CACHE


def make_in_maps(inputs):
    """Shard FULL inputs into per-core input maps (own rows rotated first).

    Weight matrices are rounded to bf16 on the host — the device consumes
    them directly with no cast instructions."""
    import ml_dtypes

    x = np.asarray(inputs["x"], dtype=np.float32)

    def cols(name):
        v = np.asarray(inputs[name], dtype=np.float32)
        return np.ascontiguousarray(v.reshape(-1, P).T)

    def brow(name):
        v = np.asarray(inputs[name], dtype=np.float32)
        return np.ascontiguousarray(np.broadcast_to(v, (P, v.shape[0])))

    bf16_names = ["Wq", "Wk", "Wv", "Wo", "Wfc", "Wproj"]
    shared = {n: cols(n) for n in
              ["ln1_w", "ln1_b", "bq", "bk", "ln2_w", "ln2_b", "bfc"]}
    shared.update({n: brow(n) for n in ["bv", "bo", "bproj"]})
    shared.update(
        {n: np.ascontiguousarray(
            np.asarray(inputs[n], dtype=np.float32).astype(ml_dtypes.bfloat16))
         for n in bf16_names}
    )
    in_maps = []
    for c in range(N_CORES):
        b, half = c // 2, c % 2
        xb = x[b]
        x_core = np.concatenate(
            [xb[half * SO : (half + 1) * SO], xb[(1 - half) * SO : (2 - half) * SO]],
            axis=0,
        )
        m = {"x": np.ascontiguousarray(x_core)}
        m.update(shared)
        in_maps.append(m)
    return in_maps


def kernel(**inputs) -> np.ndarray:
    from concourse.bass_utils import run_bass_kernel_spmd

    nc = _get_nc()
    in_maps = make_in_maps(inputs)
    res = run_bass_kernel_spmd(nc, in_maps, list(range(N_CORES)))
    B = 4
    out = np.empty((B, S, D), dtype=np.float32)
    for c in range(N_CORES):
        b, half = c // 2, c % 2
        out[b, half * SO : (half + 1) * SO] = res.results[c]["out"]
    return out
